# revision 7
# baseline (speedup 1.0000x reference)
"""ARD-RBF covariance kernel for Trainium2 (Bass/Tile), 8-core row-parallel.

Math (matches the reference):
    s  = exp(-weights[:, 0])                      # (D,) inverse lengthscales
    Us = U * s ; Vs = V * s
    sq[i, j] = ||Us_i||^2 + ||Vs_j||^2 - 2 Us_i . Vs_j
    K[i, j]  = exp(2*sn) * exp(-0.5 * max(sq, 0))

Device strategy (per core, rows sharded 8 ways):
    One augmented matmul computes sq directly in PSUM:
      lhsT (K=18 x 128) rows: [-2*s*U^T block ; ||Us||^2 row ; ones row]
      rhs  (K=18 x 512) rows: [ s*V^T        ; ones row     ; ||Vs||^2 row]
    Then a single ScalarE activation per tile computes
      out = Exp(-0.5 * psum + 2*sn)   (PSUM -> SBUF), and DMA writes out.
    Row norms are computed on-device via ones-vector matmuls.

The (8192, 8192) f32 output (256 MB) makes this memory-bound on the
HBM write (~90 us/core at ~358 GB/s); PE/ACT work is overlapped.
"""

import numpy as np

import concourse.bacc as bacc
import concourse.bass as bass  # noqa: F401  (AP helpers)
import concourse.mybir as mybir
import concourse.tile as tile

N, M, D = 8192, 8192, 16
N_CORES = 8
ROWS = N // N_CORES  # 1024 rows of U per core
P = 128              # output partitions per row block
FREE = 512           # matmul moving free dim (one PSUM bank of f32)
QUAD = 2048          # ACT chunk: 4 banks
K = D + 2            # augmented contraction dim

F32 = mybir.dt.float32
AF = mybir.ActivationFunctionType


def build_program(rows=ROWS, m_cols=M, repeats=1):
    """Build the per-core Bass program. rows/m_cols shrinkable for sim."""
    rb = rows // P
    nq = m_cols // QUAD

    nc = bacc.Bacc()
    ut = nc.declare_dram_parameter("ut", [D, rows], F32, isOutput=False)
    vt = nc.declare_dram_parameter("vt", [D, m_cols], F32, isOutput=False)
    w = nc.declare_dram_parameter("w", [D, 1], F32, isOutput=False)
    sn = nc.declare_dram_parameter("sn", [1, 1], F32, isOutput=False)
    out = nc.declare_dram_parameter("out", [rows, m_cols], F32, isOutput=True)

    with tile.TileContext(nc) as tc:
        with (
            tc.tile_pool(name="singles", bufs=1) as singles,
            tc.tile_pool(name="psum_pool", bufs=2, space="PSUM") as psum_pool,
            tc.tile_pool(name="obuf_pool", bufs=2) as obuf_pool,
        ):
            # --- scale factors -------------------------------------------
            wt = singles.tile([D, 1], F32)
            nc.sync.dma_start(wt[:], w[:])
            s_t = singles.tile([D, 1], F32)
            nc.scalar.activation(s_t[:], wt[:], AF.Exp, scale=-1.0)  # s = exp(-w)
            s2_t = singles.tile([D, 1], F32)
            nc.scalar.mul(s2_t[:], s_t[:], -2.0)                     # -2s

            snb = singles.tile([P, 1], F32)
            nc.gpsimd.dma_start(snb[:], sn[:].to_broadcast((P, 1)))
            bias2 = singles.tile([P, 1], F32)
            nc.scalar.mul(bias2[:], snb[:], 2.0)                     # 2*sn

            ones16 = singles.tile([D, 1], F32)
            nc.vector.memset(ones16[:], 1.0)
            quart16 = singles.tile([D, 1], F32)
            nc.vector.memset(quart16[:], 0.25)

            # Compute-engine SBUF APs must start at partition 0/32/64/96, so
            # the augmented rows (16, 17) are built in partition-0 scratch
            # tiles and DMA'd into place (DMA has no partition restriction).
            onesrow = singles.tile([1, m_cols], F32)
            nc.vector.memset(onesrow[:], 1.0)

            # --- lhsT: L = [-2 s U^T ; u2 ; 1] ---------------------------
            L = singles.tile([K, rows], F32)
            nc.sync.dma_start(L[0:D, :], ut[:])
            # tensor_tensor with a broadcast AP rather than tensor_scalar:
            # TensorScalarPtr only has one sync-wait slot in the ISA.
            nc.vector.tensor_mul(L[0:D, :], L[0:D, :], s2_t.to_broadcast((D, rows)))
            nc.sync.dma_start(L[D + 1 : D + 2, :], onesrow[:, :rows])
            qU = singles.tile([D, rows], F32)
            nc.vector.tensor_mul(qU[:], L[0:D, :], L[0:D, :])        # 4 s^2 U^2
            u2row = singles.tile([1, rows], F32)
            for c in range(rows // FREE):
                ps = psum_pool.tile([P, QUAD], F32, tag="ps", name="ps")
                nc.tensor.matmul(
                    ps[0:1, 0:FREE], quart16[:], qU[:, c * FREE : (c + 1) * FREE],
                    start=True, stop=True,
                )
                nc.vector.tensor_copy(
                    u2row[:, c * FREE : (c + 1) * FREE], ps[0:1, 0:FREE]
                )
            nc.sync.dma_start(L[D : D + 1, :], u2row[:])

            # --- rhs: R = [s V^T ; 1 ; v2] -------------------------------
            R = singles.tile([K, m_cols], F32)
            nc.sync.dma_start(R[0:D, :], vt[:])
            nc.vector.tensor_mul(R[0:D, :], R[0:D, :], s_t.to_broadcast((D, m_cols)))
            nc.sync.dma_start(R[D : D + 1, :], onesrow[:])
            qV = singles.tile([D, m_cols], F32)
            nc.vector.tensor_mul(qV[:], R[0:D, :], R[0:D, :])        # s^2 V^2
            v2row = singles.tile([1, m_cols], F32)
            for c in range(m_cols // FREE):
                ps = psum_pool.tile([P, QUAD], F32, tag="ps", name="ps")
                nc.tensor.matmul(
                    ps[0:1, 0:FREE], ones16[:], qV[:, c * FREE : (c + 1) * FREE],
                    start=True, stop=True,
                )
                nc.vector.tensor_copy(
                    v2row[:, c * FREE : (c + 1) * FREE], ps[0:1, 0:FREE]
                )
            nc.sync.dma_start(R[D + 1 : D + 2, :], v2row[:])

            # --- main loop ----------------------------------------------
            for _rep in range(repeats):
                for m in range(rb):
                    ob = obuf_pool.tile([P, m_cols], F32, tag="ob", name="ob")
                    for q in range(nq):
                        ps = psum_pool.tile([P, QUAD], F32, tag="ps", name="ps")
                        for k in range(QUAD // FREE):
                            n = q * (QUAD // FREE) + k
                            nc.tensor.matmul(
                                ps[:, k * FREE : (k + 1) * FREE],
                                L[:, m * P : (m + 1) * P],
                                R[:, n * FREE : (n + 1) * FREE],
                                start=True, stop=True,
                            )
                        nc.scalar.activation(
                            ob[:, q * QUAD : (q + 1) * QUAD], ps[:],
                            AF.Exp, bias=bias2[:], scale=-0.5,
                        )
                    nc.sync.dma_start(out[m * P : (m + 1) * P, :], ob[:])

    nc.compile()  # bacc lowering: splits multi-waits, reg alloc, etc.
    return nc


_PROGRAM_CACHE = {}


def get_program(rows=ROWS, m_cols=M, repeats=1):
    key = (rows, m_cols, repeats)
    if key not in _PROGRAM_CACHE:
        _PROGRAM_CACHE[key] = build_program(rows, m_cols, repeats)
    return _PROGRAM_CACHE[key]


def make_in_maps(U, V, weights, sn):
    U = np.ascontiguousarray(np.asarray(U, dtype=np.float32))
    V = np.ascontiguousarray(np.asarray(V, dtype=np.float32))
    w = np.ascontiguousarray(np.asarray(weights, dtype=np.float32).reshape(D, 1))
    snr = np.asarray(sn, dtype=np.float32).reshape(1, 1)
    vt = np.ascontiguousarray(V.T)
    in_maps = []
    for c in range(N_CORES):
        ut = np.ascontiguousarray(U[c * ROWS : (c + 1) * ROWS].T)
        in_maps.append({"ut": ut, "vt": vt, "w": w, "sn": snr})
    return in_maps


def kernel(U, V, weights, sn):
    from concourse.bass_utils import run_bass_kernel_spmd

    nc = get_program()
    in_maps = make_in_maps(U, V, weights, sn)
    res = run_bass_kernel_spmd(nc, in_maps, core_ids=list(range(N_CORES)))
    return np.concatenate([r["out"] for r in res.results], axis=0)
